# revision 62
# baseline (speedup 1.0000x reference)
"""Multi-head attention (B=4, N=2048, DIM=1024, H=16, DH=64) on 8 trn2 cores.

Sharding: data-parallel over batch (4) x tensor-parallel over heads (2 groups
of 8). Each core computes q/k/v projections for its 8 heads, attention, and a
partial output projection; the host sums the two partials per batch and adds
the bias.

Key design points (v3):
  - Host passes x PRE-TRANSPOSED (xT [DIM, N] bf16): kills the on-chip
    transpose chain (PE) and its PSUM->SBUF evacuations (DVE), and shortens
    the startup critical path.
  - Scores run as fp8e4 DoubleRow matmuls (0.5 cycles/row); q^T/k^T are
    quantized to fp8 at PSUM evacuation. Sub-tile 1 multiplies a zeroed q
    block (no partition-split layout needed).
  - exp is SPLIT between ScalarE (native Exp activation, ~0.83ns/el) and a
    custom DVE op EXP4_ANT = (((c3 z + c2) z + c1) z + 1)^4 ~ e^z (8 ALU
    stages: deg-3 Horner + 2 squarings; coefficients fold the 1/sqrt(DH)
    scale; registered at import into dve_ops.OPS). The DVE takes 2 of 8
    groups per unit (dve_groups_mode): softmax + the 2048-key averaging wash
    out the poly's worst-case ~2% tail error (measured end-to-end: rel err
    1.386e-2 vs 1.382e-2 with exact exp everywhere).
  - attn@v uses the swapped orientation: lhsT = exp tile [keys, 128 queries],
    rhs = v_aug [keys, 65] (ones column -> softmax denominators); outputs
    land as [queries, 65]; normalization is reciprocal+scale, then a PE
    transpose rebuilds attn^T for the Wo matmul (evacuated 2 query sub-tiles
    per copy).
  - v evacuations are paired 2 heads per copy for heads 2-7 (head 0/1 stay
    single: their deadlines sit at the startup filler frontier).
  - The last query block accumulates Wo m0-m2 into a bf16 partial early; the
    tail folds it back on the PE (ident @ osbP into the m3 psum group) and
    evacuates alternating ScalarE/DVE, keeping the drain off any one engine.
  - Deadline-ordered filler (pop before AND after each score emission so
    overdue producers precede their consumers) keeps both exp engines fed.

Engine busy (TimelineSim): PE 228.8us, ScalarE ~201us, DVE ~186us; runtime
275.1us (v2 baseline: 310.5us, ScalarE-bound at 267us busy). The remaining
gap to PE-busy is pss-ring WAR latency: PSUM (8 banks) only fits 2 score
buffers, so every score group exposes one exp-completion + semaphore
round-trip; deepening the ring needs banks that attn@v / projections / wo
cannot give up (bf16 PSUM matmul output, which would halve the bank cost,
is TRN3-only; fp8 k/q/v projections and fp8 attnT for the Wo matmul all
measurably exceed the 2e-2 error gate because weight-quantization noise
does not average out).
"""
import collections

import numpy as np
import ml_dtypes

import concourse.bass as bass
import concourse.mybir as mybir
import concourse.tile as tile
from concourse import bacc
from concourse.bass_utils import run_bass_kernel_spmd
from concourse.masks import make_identity

import concourse.dve_ops as dve_ops
from concourse.dve_ops import DveOp, OPS
from concourse.dve_spec import Spec, Src0, C0, C1, C2, One, sq, lower
from concourse.dve_uop import DveOpSpec

f32 = mybir.dt.float32
bf16 = mybir.dt.bfloat16
f8 = mybir.dt.float8e4
AF = mybir.ActivationFunctionType
DR = mybir.MatmulPerfMode.DoubleRow

N = 2048          # tokens
DIM = 1024        # model dim
NHL = 8           # heads per core
DH = 64           # head dim
INNER = NHL * DH  # 512 per-core inner dim
SCALE = DH ** -0.5
TB = 512          # token block
NTB = N // TB     # 4
NKT = N // 128    # 16 k-tiles
NDC = DIM // 128  # 8 dim chunks
NM = INNER // 128 # 4 inner chunks (head pairs)
NQB = 4           # query blocks (= NTB)
QB = 512

WARM = 30
OPTS = dict(
    et_bufs=20,
    fill_budget=450.0,
    early_budget=700.0,
    late_budget=700.0,
    backlog_hi=14,
    stage_bufs=6,
    dve_per_unit=2,
    dve_groups_mode="25",
    tail_units=30,
    tail_budget=1500.0,
)

# ---- custom DVE exp: (((c3 z + c2) z + c1) z + 1)^4 ~ e^z, z = s*SCALE ----
# deg-3 coefficients fit for e^z over z in [-3.75, 3.75] (relative error
# weighting for z > -1.2, absolute below); see fit_sim.py.
_EC = (0.24993133, 0.03255893, 0.0027673)
EXP_S0 = _EC[0] * SCALE
EXP_S1 = _EC[1] * SCALE * SCALE
EXP_IMM2 = _EC[2] * SCALE * SCALE * SCALE


def _make_exp_op():
    h = ((Src0 * C2 + C1) * Src0 + C0) * Src0 + One
    body = sq(sq(h))

    def ref(in0, in1, s0, s1, imm2):
        p = ((in0 * imm2 + s1) * in0 + s0) * in0 + 1.0
        return (p * p) * (p * p)

    spec = Spec(body=body, reference=ref)
    shas = {}
    for ver in ("v3", "v4"):
        ospec = DveOpSpec(name="EXP4_ANT", opcode=31,
                          uops=lower(spec, ver=ver), rd1_en=False)
        shas[ver] = ospec.sha(ver)
    return DveOp("EXP4_ANT", spec, subdim=False, uops_sha=shas)


if "EXP4_ANT" not in dve_ops._SUB_OPCODE_FOR_NAME:
    _op = _make_exp_op()
    OPS.append(_op)
    dve_ops._SUB_OPCODE_FOR_NAME[_op.name] = (
        dve_ops._CUSTOM_DVE_ROW_BASE + len(OPS) - 1)
EXP_OP = next(op for op in OPS if op.name == "EXP4_ANT")


def build_nc(**over):
    o = dict(OPTS)
    o.update(over)

    nc = bacc.Bacc(None, target_bir_lowering=False)

    xt_d = nc.dram_tensor("xt", [DIM, N], bf16, kind="ExternalInput")
    wq_d = nc.dram_tensor("wq", [DIM, INNER], bf16, kind="ExternalInput")
    wk_d = nc.dram_tensor("wk", [DIM, INNER], bf16, kind="ExternalInput")
    wv_d = nc.dram_tensor("wv", [DIM, INNER], bf16, kind="ExternalInput")
    wo_d = nc.dram_tensor("wo", [INNER, DIM], bf16, kind="ExternalInput")
    out_d = nc.dram_tensor("out", [N, DIM], f32, kind="ExternalOutput")

    xt_v = xt_d.rearrange("(c k) n -> k c n", k=128)   # [128, 8, 2048]
    wq_v = wq_d.rearrange("(c k) n -> k c n", k=128)   # [128, 8, 512]
    wk_v = wk_d.rearrange("(c k) n -> k c n", k=128)
    wv_v = wv_d.rearrange("(c k) n -> k c n", k=128)
    wo_v = wo_d.rearrange("(c k) n -> k c n", k=128)   # [128, 4, 1024]

    with tile.TileContext(nc) as tc:
        with (
            tc.tile_pool(name="consts", bufs=1) as consts,
            tc.tile_pool(name="wsb", bufs=1) as wsb,
            tc.tile_pool(name="kqv", bufs=1) as kqv,
            tc.tile_pool(name="etp", bufs=o["et_bufs"]) as etp,
            tc.tile_pool(name="stp", bufs=o["stage_bufs"]) as stp,
            tc.tile_pool(name="attnp", bufs=4) as attnp,
            tc.tile_pool(name="outp", bufs=o["outp_bufs"]) as outp,
            tc.tile_pool(name="ps_s", bufs=2, space="PSUM") as ps_s,
            tc.tile_pool(name="ps_av", bufs=2, space="PSUM") as ps_av,
            tc.tile_pool(name="ps_f", bufs=2, space="PSUM") as ps_f,
        ):
            ident = consts.tile([128, 128], bf16)
            make_identity(nc, ident)

            # preload the Exp activation table immediately
            dummy = consts.tile([128, 1], f32)
            nc.scalar.activation(out=dummy, in_=ident[:, 0:1], func=AF.Exp)

            # keep the PE p-state ramp warm until real work arrives
            for _ in range(o.get('warm', WARM)):
                scratch = ps_s.tile([128, 128], bf16, name="warm", tag="s")
                nc.tensor.transpose(scratch, ident, ident)

            kT8 = kqv.tile([128, NM, NTB, TB + 128], f8)     # fp8 k^T, per-tb pad
            qT8 = kqv.tile([128, NTB, NM, 2, QB], f8)        # fp8 q^T + zeros
            v_sb = kqv.tile([128, NKT, NHL, DH + 1], bf16)   # v + ones col

            # one-time zero/one fills on gpsimd (idle engine); the slices
            # needed by unit 0's scores (m0/qb0) go first so the Pool queue
            # never gates the first DR matmuls
            nc.gpsimd.memset(qT8[:, 0, 0, 1, :], 0.0)
            nc.gpsimd.memset(kT8[:, 0, :, TB:TB + 128], 0.0)
            nc.gpsimd.memset(qT8[:, 1:, 0, 1, :], 0.0)
            nc.gpsimd.memset(v_sb[:, :, :, DH], 1.0)
            nc.gpsimd.memset(kT8[:, 1:, :, TB:TB + 128], 0.0)
            nc.gpsimd.memset(qT8[:, :, 1:, 1, :], 0.0)

            # ---- DMA: xT per token-block; weight m0 chunks early ----
            wk_sb = wsb.tile([128, NDC, INNER], bf16)
            wq_sb = wsb.tile([128, NDC, INNER], bf16)
            wv_sb = wsb.tile([128, NDC, INNER], bf16)
            wo_sb = wsb.tile([128, NM, DIM], bf16)
            xTs = []
            for tb in range(NTB):
                xTs.append(wsb.tile([128, NDC, TB], bf16, name=f"xT{tb}"))

            def dma_xt(tb, half):
                sl = slice(tb * TB + half * 256, tb * TB + half * 256 + 256)
                nc.sync.dma_start(xTs[tb][:, :, half * 256:half * 256 + 256],
                                  xt_v[:, :, sl])

            def dma_w(sb_t, view, m0, m1):
                nc.sync.dma_start(sb_t[:, :, m0 * 128:m1 * 128],
                                  view[:, :, m0 * 128:m1 * 128])

            dma_w(wk_sb, wk_v, 0, 1)
            dma_xt(0, 0)
            dma_xt(0, 1)
            dma_w(wq_sb, wq_v, 0, 1)
            nc.sync.dma_start(xTs[1], xt_v[:, :, TB:2 * TB])
            nc.sync.dma_start(xTs[2], xt_v[:, :, 2 * TB:3 * TB])
            nc.sync.dma_start(xTs[3], xt_v[:, :, 3 * TB:4 * TB])
            dma_w(wv_sb, wv_v, 0, 1)
            dma_w(wk_sb, wk_v, 1, 4)
            dma_w(wv_sb, wv_v, 1, 4)
            dma_w(wq_sb, wq_v, 1, 4)
            nc.sync.dma_start(wo_sb, wo_v)

            # ---------------- projection groups ----------------
            emitted = collections.Counter()   # producer completion tracking

            def udx(h, qb):
                return (h // 2) * 8 + qb * 2 + (h % 2)

            def kq0_chain():
                """k then q m0 projections for tb0 (not interleaved: the k
                chain only needs wk+xt0 and runs during the wq DMA; the
                interleaved form stalled the whole chain on wq)."""
                psk = ps_f.tile([128, TB], f32, name="pskc", tag="f")
                for dc in range(NDC):
                    nc.tensor.matmul(psk, wk_sb[:, dc, 0:128],
                                     xTs[0][:, dc, :],
                                     start=(dc == 0), stop=(dc == NDC - 1))
                nc.vector.tensor_copy(kT8[:, 0, 0, 0:TB], psk)
                psq = ps_f.tile([128, TB], f32, name="psqc", tag="f")
                for dc in range(NDC):
                    nc.tensor.matmul(psq, wq_sb[:, dc, 0:128],
                                     xTs[0][:, dc, :],
                                     start=(dc == 0), stop=(dc == NDC - 1))
                nc.vector.tensor_copy(qT8[:, 0, 0, 0, :], psq)
                emitted.update([("k", 0), ("q", 0, 0)])

            def kq_steps(w_sb, tb, m, evac):
                """Four steps of 2 matmuls; evac(psum) runs on the last.
                The psum group stays open between steps, so the pop
                machinery must not emit other ps_f tiles in between."""
                cell = {}

                def quarter(qtr, w):
                    if qtr == 0:
                        cell["ps"] = ps_f.tile([128, TB], f32, name="pskq",
                                               tag="f")
                    for dc in range(w * qtr, w * qtr + w):
                        nc.tensor.matmul(
                            cell["ps"], w_sb[:, dc, m * 128:m * 128 + 128],
                            xTs[tb][:, dc, :],
                            start=(dc == 0), stop=(dc == NDC - 1))
                    if (qtr + 1) * w == NDC:
                        evac(cell["ps"])
                if o["kq_step8"]:
                    return [(o["kq_cost"], lambda q=q: quarter(q, 1))
                            for q in range(7)] + \
                           [(o["kq_cost"] + 670, lambda: quarter(7, 1))]
                return [(470, lambda q=q: quarter(q, 2)) for q in range(3)] + \
                       [(1140, lambda: quarter(3, 2))]

            def _evac(dst, ps):
                if o["kq_evac"] == "scalar":
                    nc.scalar.copy(dst, ps)
                else:
                    nc.vector.tensor_copy(dst, ps)

            def k_steps(tb, m):
                return kq_steps(
                    wk_sb, tb, m,
                    lambda ps: _evac(kT8[:, m, tb, 0:TB], ps))

            def q_steps(tb, m):
                return kq_steps(
                    wq_sb, tb, m,
                    lambda ps: _evac(qT8[:, tb, m, 0, :], ps))

            def k_group(tb, m):
                for _, fn in k_steps(tb, m):
                    fn()

            def v_group(tb, ts, h, nh=1, j=0, half=None, cell={}):
                """v for nh heads x 128 tokens; shared psum, one evac.
                half=0/1 emits only that dc-half (finer filler steps)."""
                if j == 0 and half in (None, 0):
                    cell["ps"] = ps_f.tile([128, nh, DH], f32, name="psv",
                                           tag="f")
                psv = cell["ps"]
                dcs = range(NDC) if half is None else \
                    range(half * 4, half * 4 + 4)
                for dc in dcs:
                    nc.tensor.matmul(
                        psv[:, j, :], xTs[tb][:, dc, ts * 128:ts * 128 + 128],
                        wv_sb[:, dc, (h + j) * DH:(h + j + 1) * DH],
                        start=(dc == 0), stop=(dc == NDC - 1))
                if j == nh - 1 and half in (None, 1):
                    kt = tb * (TB // 128) + ts
                    nc.vector.tensor_copy(
                        v_sb[:, kt, h:h + nh, 0:DH], psv)

            # ---------------- Phase A ----------------
            kq0_chain()

            # ---------------- filler (deadline order) ----------------
            filler = []
            fill_state = {"cur": None, "i": 0}

            def add_k(m, tbs=(0, 1, 2, 3)):
                for tb in tbs:
                    if m == 0:
                        dl = 2 * tb
                    else:
                        dl = 64 * m - 14 + 4 * tb
                    steps = k_steps(tb, m) + \
                        [(0, lambda: emitted.update([("k", m)]))]
                    filler.append((dl, steps))

            def add_q(tb, m):
                dl = 8 * udx(2 * m, tb) - 2
                steps = q_steps(tb, m) + \
                    [(0, lambda: emitted.update([("q", tb, m)]))]
                filler.append((dl, steps))

            def add_v(h, nh=1):
                # consumed by replay of unit udx(h, 0), emitted at that
                # unit + 1 (or +2 with startup backlog); never force before
                # unit 1. nh=2 pairs heads h,h+1 (one evac) for units whose
                # deadlines sit well behind the filler frontier.
                dl0 = 8 * udx(h, 0) + 8
                for tb in range(NTB):
                    for ts in range(TB // 128):
                        kt = tb * 4 + ts
                        if o["v_step_half"]:
                            steps = [(155, lambda tb=tb, ts=ts, j=j, hf=hf:
                                      v_group(tb, ts, h, nh, j, hf))
                                     for j in range(nh) for hf in (0, 1)]
                        else:
                            steps = [(o["v_cost"], lambda tb=tb, ts=ts, j=j:
                                      v_group(tb, ts, h, nh, j))
                                     for j in range(nh)]
                        steps.append((0, lambda: emitted.update(
                            [("v", h + j) for j in range(nh)])))
                        filler.append((dl0 + kt // 4, steps))

            # unit order: head pairs sharing an m-chunk, m-major.
            add_k(0, tbs=(1, 2, 3))   # tb0 m0 emitted in phase A
            if o["v01_pair"]:
                add_v(0, nh=2)
            else:
                add_v(0)              # by u1
                add_v(1)              # by u2
            for tb in (1, 2, 3):
                add_q(tb, 0)          # by u2/u4/u6
            for m in (1, 2, 3):
                add_k(m)              # by u8m
                add_q(0, m)
                add_v(2 * m, nh=2)    # paired: one evac per 2 heads
                add_q(1, m)
                add_q(2, m)
                add_q(3, m)

            slot_now = [0]
            filler_sorted = [False]

            def pop_filler(budget):
                if not filler_sorted[0]:
                    filler.sort(key=lambda e: e[0])
                    filler_sorted[0] = True
                spent = 0.0
                while True:
                    if fill_state["cur"] is None:
                        overdue = bool(filler) and \
                            filler[0][0] <= slot_now[0]
                        if not filler or (spent >= budget and not overdue):
                            return
                        dl, steps = filler.pop(0)
                        fill_state["cur"] = steps
                        fill_state["dl"] = dl
                        fill_state["i"] = 0
                    else:
                        overdue = fill_state["dl"] <= slot_now[0]
                        if spent >= budget and not overdue:
                            return   # yield mid-entry; resume next slot
                    steps = fill_state["cur"]
                    cost, fn = steps[fill_state["i"]]
                    fn()
                    spent += cost
                    fill_state["i"] += 1
                    if fill_state["i"] >= len(steps):
                        fill_state["cur"] = None

            # ---------------- Phase B ----------------
            units = [(2 * m + hh, qb)
                     for m in range(NM) for qb in range(NQB)
                     for hh in range(2)]

            attnT = [attnp.tile([128, NM, QB], bf16, name=f"attnT{qb}",
                                tag="attnT") for qb in range(NQB)]

            def wo_unit(qb, qs, d):
                psf = ps_f.tile([128, 512], f32, name=f"psf{qs}{d}", tag="f")
                for m in range(NM):
                    nc.tensor.matmul(
                        psf, attnT[qb][:, m, qs * 128:qs * 128 + 128],
                        wo_sb[:, m, d * 512:d * 512 + 512],
                        start=(m == 0), stop=(m == NM - 1))
                osb = wo_unit.osbs.get((qb, qs))
                if osb is None:
                    osb = outp.tile([128, DIM], f32, name=f"osb{qs}",
                                    tag="osb")
                    wo_unit.osbs[(qb, qs)] = osb
                nc.vector.tensor_copy(osb[:, d * 512:d * 512 + 512], psf)
                if d == 1:
                    r0 = qb * QB + qs * 128
                    nc.sync.dma_start(out_d[r0:r0 + 128, :], osb)
                    del wo_unit.osbs[(qb, qs)]
            wo_unit.osbs = {}

            def wo_half(qb, qs, d, h, cell={}):
                if h == 0:
                    cell["ps"] = ps_f.tile([128, 512], f32,
                                           name=f"psf{qs}{d}", tag="f")
                psf = cell["ps"]
                for m in (2 * h, 2 * h + 1):
                    nc.tensor.matmul(
                        psf, attnT[qb][:, m, qs * 128:qs * 128 + 128],
                        wo_sb[:, m, d * 512:d * 512 + 512],
                        start=(m == 0), stop=(m == NM - 1))
                if h == 1:
                    osb = wo_unit.osbs.get((qb, qs))
                    if osb is None:
                        osb = outp.tile([128, DIM], f32, name=f"osb{qs}",
                                        tag="osb")
                        wo_unit.osbs[(qb, qs)] = osb
                    nc.vector.tensor_copy(osb[:, d * 512:d * 512 + 512], psf)
                    if d == 1:
                        r0 = qb * QB + qs * 128
                        nc.sync.dma_start(out_d[r0:r0 + 128, :], osb)
                        del wo_unit.osbs[(qb, qs)]

            def add_wo(qb):
                for qs in range(4):
                    for d in range(2):
                        if o["wo_split"]:
                            filler.append(
                                (10 ** 9,
                                 [(440, lambda qs=qs, d=d: wo_half(qb, qs, d, 0)),
                                  (440, lambda qs=qs, d=d: wo_half(qb, qs, d, 1))]))
                        else:
                            filler.append(
                                (10 ** 9,
                                 [(o["wo_cost"],
                                   lambda qs=qs, d=d: wo_unit(qb, qs, d))]))

            # last query block: m0-m2 accumulate early into a bf16 partial;
            # only the final head-pair's matmul stays in the tail
            QBL = NQB - 1
            osbP = {}

            def wo_partial(qs, d):
                psf = ps_f.tile([128, 512], f32, name=f"psfP{qs}{d}",
                                tag="f")
                for m in range(NM - 1):
                    nc.tensor.matmul(
                        psf, attnT[QBL][:, m, qs * 128:qs * 128 + 128],
                        wo_sb[:, m, d * 512:d * 512 + 512],
                        start=(m == 0), stop=(m == NM - 2))
                t = osbP.get(qs)
                if t is None:
                    t = outp.tile([128, DIM], bf16, name=f"osbP{qs}",
                                  tag="osbP", bufs=4)
                    osbP[qs] = t
                nc.vector.tensor_copy(t[:, d * 512:d * 512 + 512], psf)

            def add_wo_partial():
                for qs in range(4):
                    for d in range(2):
                        filler.append(
                            (10 ** 9,
                             [(700, lambda qs=qs, d=d: wo_partial(qs, d))]))

            # per-unit state
            ustate = {}   # u_idx -> dict(ets, psos, stages, h, qb)

            def dve_groups(u_idx):
                mode = o["dve_groups_mode"]
                if u_idx < o["dve_early_units"]:
                    mode = o["dve_early_mode"]
                elif u_idx >= 32 - o["dve_last_units"]:
                    mode = o["dve_last_mode"]
                return {int(c) for c in mode}

            def emit_scores_exp(u_idx, g):
                h, qb = units[u_idx]
                po = (h % 2) * 64
                m = h // 2
                if u_idx > 0:
                    assert emitted[("k", m)] == NTB, (u_idx, m)
                    assert (qb, m) == (0, 0) or \
                        emitted[("q", qb, m)] == 1, (u_idx, qb, m)
                else:
                    assert emitted[("k", 0)] >= g // 2 + 1, (g,)
                st = ustate[u_idx]
                pss = ps_s.tile([128, 2, QB], f32, name=f"pss{g}", tag="s")
                for i in range(2):
                    kt = 2 * g + i
                    c0 = (kt % 4) * 128
                    nc.tensor.matmul(
                        pss[:, i, :],
                        kT8[po:po + 64, m, kt // 4, c0:c0 + 256].rearrange(
                            "p (s f) -> p s f", s=2),
                        qT8[po:po + 64, qb, m, :, :],
                        start=True, stop=True, perf_mode=DR)
                et = etp.tile([128, 2, QB], bf16, name=f"et{g}", tag="et")
                if g in dve_groups(u_idx):
                    if o["dve_split"]:
                        for i in range(2):
                            nc.vector._custom_dve(
                                EXP_OP, out=et[:, i, :], in0=pss[:, i, :],
                                s0=EXP_S0, s1=EXP_S1, imm2=EXP_IMM2)
                    else:
                        nc.vector._custom_dve(
                            EXP_OP,
                            out=et.rearrange("p a b -> p (a b)"),
                            in0=pss.rearrange("p a b -> p (a b)"),
                            s0=EXP_S0, s1=EXP_S1, imm2=EXP_IMM2)
                else:
                    nc.scalar.activation(out=et, in_=pss, func=AF.Exp,
                                         scale=SCALE)
                st["ets"].append(et)

            def replay_qs(u_idx, qs, pool=None):
                """attn@v for one query sub-tile of a finished unit."""
                h, qb = units[u_idx]
                assert emitted[("v", h)] == NKT, (u_idx, h)
                st = ustate[u_idx]
                pool = pool or ps_av
                pso = pool.tile([128, 512], f32, name=f"pso{qs}",
                                tag="av" if pool is ps_av else "s")
                st["psos"][qs] = pso
                for g in range(8):
                    et = st["ets"][g]
                    for i in range(2):
                        kt = 2 * g + i
                        nc.tensor.matmul(
                            pso[:, 0:DH + 1],
                            et[:, i, qs * 128:qs * 128 + 128],
                            v_sb[:, kt, h, :],
                            start=(kt == 0), stop=(kt == NKT - 1))

            def norm_qs(u_idx, qs):
                h, qb = units[u_idx]
                st = ustate[u_idx]
                pso = st["psos"][qs]
                recip = stp.tile([128, 1], f32, name=f"rc{qs}", tag="rc")
                nc.vector.reciprocal(recip, pso[:, DH:DH + 1])
                stage = stp.tile([128, DH], bf16, name=f"st{qs}", tag="st")
                nc.vector.tensor_scalar_mul(stage, pso[:, 0:DH], recip)
                st["stages"][qs] = stage

            def trans_pair(u_idx, qs0):
                """transpose 2 query sub-tiles + one batched evacuation."""
                h, qb = units[u_idx]
                po = (h % 2) * 64
                m = h // 2
                st = ustate[u_idx]
                pool, tg = (ps_f, "f") if o["ptt_pool"] == "f" else (ps_av, "av")
                ptT = pool.tile([64, 2, 128], bf16, name=f"ptT{qs0}",
                                tag=tg)
                for j in range(2):
                    nc.tensor.transpose(ptT[:, j, :],
                                        st["stages"][qs0 + j], ident)
                nc.vector.tensor_copy(
                    attnT[qb][po:po + 64, m,
                              qs0 * 128:qs0 * 128 + 256].rearrange(
                        "p (a b) -> p a b", a=2), ptT)

            def post_step(u_idx, g):
                """One step of a finished unit's post-processing."""
                h, qb = units[u_idx]
                if g == 0:
                    replay_qs(u_idx, 0)
                elif g == 1:
                    replay_qs(u_idx, 1)
                elif g == 2:
                    norm_qs(u_idx, 0)
                    replay_qs(u_idx, 2)
                elif g == 3:
                    norm_qs(u_idx, 1)
                    replay_qs(u_idx, 3)
                elif g == 4:
                    norm_qs(u_idx, 2)
                    trans_pair(u_idx, 0)
                elif g == 5:
                    norm_qs(u_idx, 3)
                elif g == 6:
                    trans_pair(u_idx, 2)
                    if h == 7 and qb != QBL:
                        add_wo(qb)
                    if h == 5 and qb == QBL:
                        add_wo_partial()
                    ustate.pop(u_idx, None)

            pending_posts = []   # (u_idx, step) queue; self-healing backlog

            def run_posts():
                steps = 2 if len(pending_posts) > o["backlog_hi"] else 1
                for _ in range(steps):
                    if not pending_posts:
                        return
                    u_i, st_i = pending_posts.pop(0)
                    post_step(u_i, st_i)

            for u_idx in range(len(units)):
                ustate[u_idx] = dict(ets=[], psos={}, stages={})
                if u_idx == 0:
                    budget = 0.0       # keep unit 0's PE path clean
                elif u_idx < 3:
                    budget = o["early_budget"]
                elif u_idx >= o["tail_units"]:
                    budget = o["tail_budget"]
                elif u_idx >= 24:
                    budget = o["late_budget"]
                elif o["boost_lo"] <= u_idx < o["boost_hi"]:
                    budget = o["boost_budget"]
                else:
                    budget = o["fill_budget"]
                # delay unit 0's post by one unit (its deps arrive late);
                # the queue self-heals the backlog by unit 3.
                if u_idx == 2:
                    pending_posts.extend((0, st) for st in range(7))
                if u_idx >= 2:
                    pending_posts.extend((u_idx - 1, st) for st in range(7))
                for g in range(8):
                    slot_now[0] = 8 * u_idx + g
                    pop_filler(0.0)   # force overdue producers first
                    emit_scores_exp(u_idx, g)
                    if o["posts_first"]:
                        run_posts()
                        pop_filler(budget)
                    elif o["posts_split"]:
                        run_posts()
                        pop_filler(budget)
                        run_posts()
                    else:
                        pop_filler(budget)
                        run_posts()

            # ---------------- tail (pipelined per query sub-tile) ----------
            slot_now[0] = 10 ** 9 - 1
            while pending_posts:
                u_i, st_i = pending_posts.pop(0)
                post_step(u_i, st_i)
            while filler or fill_state["cur"] is not None:
                pop_filler(1e9)
            last = len(units) - 1
            replay_qs(last, 0)
            replay_qs(last, 1)
            replay_qs(last, 2, pool=ps_s)
            replay_qs(last, 3, pool=ps_s)
            for qs in range(4):
                norm_qs(last, qs)
            trans_pair(last, 0)
            trans_pair(last, 2)
            for qs in range(4):
                for d in range(2):
                    psf = ps_f.tile([128, 512], f32, name=f"psfL{qs}{d}",
                                    tag="f")
                    nc.tensor.matmul(
                        psf, attnT[QBL][:, NM - 1, qs * 128:qs * 128 + 128],
                        wo_sb[:, NM - 1, d * 512:d * 512 + 512],
                        start=True, stop=False)
                    # fold the bf16 m0-m2 partial in on the PE (ident @ osbP)
                    # and evacuate on ScalarE, keeping the tail off DVE
                    nc.tensor.matmul(
                        psf, ident, osbP[qs][:, d * 512:d * 512 + 512],
                        start=False, stop=True)
                    fin = outp.tile([128, 512], f32, name=f"fin{qs}{d}",
                                    tag="fin", bufs=4)
                    if (qs * 2 + d) % 2 == 0:
                        nc.scalar.copy(fin, psf)
                    else:
                        nc.vector.tensor_copy(fin, psf)
                    r0 = QBL * QB + qs * 128
                    nc.sync.dma_start(
                        out_d[r0:r0 + 128, d * 512:d * 512 + 512], fin)

    nc.compile()
    return nc


_NC = None


def _get_nc():
    global _NC
    if _NC is None:
        _NC = build_nc()
    return _NC


def kernel(x, Wq, Wk, Wv, Wo, bo):
    x = np.asarray(x, dtype=np.float32)
    Wq = np.asarray(Wq, dtype=np.float32)
    Wk = np.asarray(Wk, dtype=np.float32)
    Wv = np.asarray(Wv, dtype=np.float32)
    Wo = np.asarray(Wo, dtype=np.float32)
    bo = np.asarray(bo, dtype=np.float32)

    B = x.shape[0]
    bf = ml_dtypes.bfloat16
    nc = _get_nc()
    in_maps = []
    for c in range(8):
        b, hh = c // 2, c % 2
        sl = slice(hh * INNER, hh * INNER + INNER)
        in_maps.append({
            "xt": np.ascontiguousarray(x[b].T.astype(bf)),
            "wq": np.ascontiguousarray(Wq[:, sl].astype(bf)),
            "wk": np.ascontiguousarray(Wk[:, sl].astype(bf)),
            "wv": np.ascontiguousarray(Wv[:, sl].astype(bf)),
            "wo": np.ascontiguousarray(Wo[sl, :].astype(bf)),
        })
    res = run_bass_kernel_spmd(nc, in_maps, core_ids=list(range(8)))
    out = np.empty((B, N, DIM), dtype=np.float32)
    for b in range(B):
        out[b] = res.results[2 * b]["out"] + res.results[2 * b + 1]["out"] + bo
    return out
